# revision 7
# baseline (speedup 1.0000x reference)
"""Trainium2 Bass kernel for the MEMOL MoE-attention block (nn_MEMOL_48052094107931).

Computation (per token row x of length 256):
  h1 = LN(x, g1, b1)
  gate = softmax(h1 @ gate_w + gate_b); comb = top2-masked gate probs (not renorm.)
  s  = sum_e comb_e * (h1 @ Wfold_e)           # qkv folded: q=k=v alias => one 256x256/expert
  A  = softmax(scale * s s^T) per token (4x4 over heads)
  out_nat[h*64+d] = (A s)[h, d]                # torch transpose(1,2).reshape folded into proj
  x2 = x + out_nat @ proj_w_perm + proj_b
  y  = x2 + gelu(LN(x2,g2,b2) @ fc1_w + fc1_b) @ fc2_w + fc2_b

Strategy: pure data-parallel over the batch across 8 NeuronCores (no collectives).
One NEFF, SPMD via run_bass_kernel_spmd. Token-major layout on chip; activations
transposed on the PE (with identity) where a matmul needs them as stationary lhsT.
Matmuls run as float32r (full fp32 data; fast PE mode for free-dim >= 256).
"""

import os
from contextlib import ExitStack

import numpy as np

import concourse.bass as bass
import concourse.mybir as mybir
import concourse.tile as tile
from concourse import bacc
from concourse.bass import ts
from concourse.bass_utils import run_bass_kernel_spmd
from concourse.masks import make_identity

F32 = mybir.dt.float32
F32R = mybir.dt.float32r
AF = mybir.ActivationFunctionType
ALU = mybir.AluOpType
AX = mybir.AxisListType

DIM = 256
E = 4
H = 4
DH = 64
HID = 1024
SCALE = DH ** -0.5
NCORES = 8
B_FULL = 65536
P = 128
EPS = 1e-5

# Matmul dtype for PE (float32r = fast fp32 mode; flip to F32 if accuracy fails).
MM_DT = F32 if os.environ.get("KERNEL_MM_F32", "0") == "1" else F32R


def _mm(nc, out, lhsT, rhs, **kw):
    nc.tensor.matmul(out, lhsT.bitcast(MM_DT), rhs.bitcast(MM_DT), **kw)


def build_program(nt, flags):
    """Builds the per-core program for nt tiles of 128 tokens each."""
    ntok = nt * P
    nc = bacc.Bacc(
        "TRN2", target_bir_lowering=False, debug=False, enable_asserts=False
    )

    x_d = nc.dram_tensor("x", [ntok, DIM], F32, kind="ExternalInput").ap()
    catw_d = nc.dram_tensor("catw", [2, P, 1028], F32, kind="ExternalInput").ap()
    projp_d = nc.dram_tensor("projp", [2, P, DIM], F32, kind="ExternalInput").ap()
    fc1w_d = nc.dram_tensor("fc1w", [2, 8, P, P], F32, kind="ExternalInput").ap()
    fc2w_d = nc.dram_tensor("fc2w", [8, P, DIM], F32, kind="ExternalInput").ap()
    out_d = nc.dram_tensor("out", [ntok, DIM], F32, kind="ExternalOutput").ap()
    # optional bias rows (only materialized when nonzero in the actual inputs)
    b1_d = b2_d = pb_d = f2b_d = gb_d = f1b_d = None
    if flags["b1"]:
        b1_d = nc.dram_tensor("b1row", [1, DIM], F32, kind="ExternalInput").ap()
    if flags["b2"]:
        b2_d = nc.dram_tensor("b2row", [1, DIM], F32, kind="ExternalInput").ap()
    if flags["pb"]:
        pb_d = nc.dram_tensor("pbrow", [1, DIM], F32, kind="ExternalInput").ap()
    if flags["f2b"]:
        f2b_d = nc.dram_tensor("f2brow", [1, DIM], F32, kind="ExternalInput").ap()
    if flags["gb"]:
        gb_d = nc.dram_tensor("gbrow", [1, E], F32, kind="ExternalInput").ap()
    if flags["f1b"]:
        f1b_d = nc.dram_tensor("f1bcol", [HID, 1], F32, kind="ExternalInput").ap()

    xv = x_d.rearrange("(n p) d -> n p d", p=P)
    ov = out_d.rearrange("(n p) d -> n p d", p=P)

    with tile.TileContext(nc) as tc, ExitStack() as ctx:
        singles = ctx.enter_context(tc.tile_pool(name="singles", bufs=1))

        ident = singles.tile([P, P], F32)
        make_identity(nc, ident[:])
        eps_t = singles.tile([P, 1], F32)
        nc.vector.memset(eps_t[:], EPS)

        # weights live as float32r (PE fast-fp32 mode needs producers that
        # round): DMA into f32 staging, cast-copy into the f32r tiles
        catw = singles.tile([P, 2, 1028], MM_DT)
        projp = singles.tile([P, 2, DIM], MM_DT)
        fc1w = singles.tile([P, 2, 8, P], MM_DT)
        fc2w = singles.tile([P, 8, DIM], MM_DT)
        with tc.tile_pool(name="wstage", bufs=1) as wstage:
            for (dst, src) in [
                (catw, catw_d.transpose([1, 0, 2])),
                (projp, projp_d.transpose([1, 0, 2])),
                (fc1w, fc1w_d.transpose([2, 0, 1, 3])),
                (fc2w, fc2w_d.transpose([1, 0, 2])),
            ]:
                stg = wstage.tile(list(dst.shape), F32, tag="stg", name="stg")
                nc.sync.dma_start(out=stg[:], in_=src)
                nc.vector.tensor_copy(dst[:], stg[:])

        def brow(name, d_ap, width):
            t = singles.tile([P, width], F32)
            nc.sync.dma_start(out=t[:], in_=d_ap.partition_broadcast(P).squeeze(1))
            return t

        b1_t = brow("b1", b1_d, DIM) if flags["b1"] else None
        b2_t = brow("b2", b2_d, DIM) if flags["b2"] else None
        pb_t = brow("pb", pb_d, DIM) if flags["pb"] else None
        f2b_t = brow("f2b", f2b_d, DIM) if flags["f2b"] else None
        gb_t = brow("gb", gb_d, E) if flags["gb"] else None
        f1b_t = None
        if flags["f1b"]:
            f1b_t = singles.tile([P, 8, 1], F32)
            nc.sync.dma_start(
                out=f1b_t[:], in_=f1b_d.rearrange("(c p) o -> p c o", p=P)
            )

        # resident activations
        xpool = ctx.enter_context(tc.tile_pool(name="xtiles", bufs=nt))
        h2pool = ctx.enter_context(tc.tile_pool(name="h2tiles", bufs=(nt + 1) // 2))
        xt = []
        for t in range(nt):
            xt.append(xpool.tile([P, DIM], F32, tag="x", name=f"x{t}"))
            nc.sync.dma_start(out=xt[t][:], in_=xv[t])
        h2g = [
            h2pool.tile([P, 2, 2 * P], MM_DT, tag="h2", name=f"h2g{g}")
            for g in range((nt + 1) // 2)
        ]

        # ---------------- pass A ----------------
        with ExitStack() as actx:
            wk = actx.enter_context(tc.tile_pool(name="wk", bufs=3))
            wks = actx.enter_context(tc.tile_pool(name="wks", bufs=4))
            ps_tr = actx.enter_context(
                tc.tile_pool(name="ps_tr", bufs=2, space="PSUM")
            )
            ps_y = actx.enter_context(tc.tile_pool(name="ps_y", bufs=1, space="PSUM"))
            ps_z = actx.enter_context(tc.tile_pool(name="ps_z", bufs=2, space="PSUM"))
            ps_at = actx.enter_context(
                tc.tile_pool(name="ps_at", bufs=2, space="PSUM")
            )

            for t in range(nt):
                x_t = xt[t][:]
                # LN1: bn_stats/aggr -> mean, biased var
                st6 = wks.tile([P, 6], F32, tag="st6")
                nc.vector.bn_stats(st6[:], x_t)
                mv = wks.tile([P, 2], F32, tag="mv")
                nc.vector.bn_aggr(mv[:], st6[:])
                # rstd = exp(-0.5*ln(var+eps))
                rstd = wks.tile([P, 1], F32, tag="rstd")
                nc.scalar.activation(rstd[:], mv[:, 1:2], AF.Ln, bias=eps_t[:])
                nc.scalar.activation(rstd[:], rstd[:], AF.Exp, scale=-0.5)
                h1 = wk.tile([P, DIM], F32, tag="h1")
                nc.vector.tensor_scalar(
                    out=h1[:], in0=x_t, scalar1=mv[:, 0:1], scalar2=rstd[:],
                    op0=ALU.subtract, op1=ALU.mult,
                )
                if b1_t is not None:
                    nc.vector.tensor_add(h1[:], h1[:], b1_t[:])

                # transpose h1 -> h1T (PE, identity), then PSUM->SBUF
                h1T_ps = ps_tr.tile([P, 2, P], F32, tag="tr")
                nc.tensor.transpose(h1T_ps[:, 0, :], h1[:, 0:P], ident[:])
                nc.tensor.transpose(h1T_ps[:, 1, :], h1[:, P:DIM], ident[:])
                h1T = wk.tile([P, 2, P], MM_DT, tag="h1T")
                nc.scalar.copy(h1T[:], h1T_ps[:])

                # qkv (+gate logits) matmuls: y[tok, e*256+f], z[tok, e]
                y_ps = ps_y.tile([P, HID], F32, tag="y")
                z_ps = ps_z.tile([P, E], F32, tag="z")
                for c in range(2):
                    kw = dict(start=(c == 0), stop=(c == 1))
                    _mm(nc, y_ps[:, 0:512], h1T[:, c, :], catw[:, c, 0:512], **kw)
                    _mm(nc, y_ps[:, 512:1024], h1T[:, c, :], catw[:, c, 512:1024], **kw)
                    _mm(nc, z_ps[:], h1T[:, c, :], catw[:, c, 1024:1028], **kw)

                # gate: softmax + top2 mask (values kept, not renormalized)
                if gb_t is not None:
                    nc.vector.tensor_add(z_ps[:], z_ps[:], gb_t[:])
                m1 = wks.tile([P, 1], F32, tag="m1")
                nc.vector.reduce_max(m1[:], z_ps[:], axis=AX.X)
                keq = wks.tile([P, E], F32, tag="keq")
                nc.vector.tensor_scalar(
                    out=keq[:], in0=z_ps[:], scalar1=m1[:], scalar2=None,
                    op0=ALU.is_equal,
                )
                zmsk = wks.tile([P, E], F32, tag="zmsk")
                nc.vector.scalar_tensor_tensor(
                    out=zmsk[:], in0=keq[:], scalar=-1e30, in1=z_ps[:],
                    op0=ALU.mult, op1=ALU.add,
                )
                m2 = wks.tile([P, 1], F32, tag="m2")
                nc.vector.reduce_max(m2[:], zmsk[:], axis=AX.X)
                zs = wks.tile([P, E], F32, tag="zs")
                nc.vector.tensor_scalar(
                    out=zs[:], in0=z_ps[:], scalar1=m1[:], scalar2=None,
                    op0=ALU.subtract,
                )
                ez = wks.tile([P, E], F32, tag="ez")
                nc.scalar.activation(ez[:], zs[:], AF.Exp)
                zsum = wks.tile([P, 1], F32, tag="zsum")
                nc.vector.reduce_sum(zsum[:], ez[:], axis=AX.X)
                rz = wks.tile([P, 1], F32, tag="rz")
                nc.vector.reciprocal(rz[:], zsum[:])
                keep = wks.tile([P, E], F32, tag="keep")
                nc.vector.tensor_scalar(
                    out=keep[:], in0=z_ps[:], scalar1=m2[:], scalar2=None,
                    op0=ALU.is_ge,
                )
                comb = wks.tile([P, E], F32, tag="comb")
                nc.vector.scalar_tensor_tensor(
                    out=comb[:], in0=ez[:], scalar=rz[:], in1=keep[:],
                    op0=ALU.mult, op1=ALU.mult,
                )

                # s = sum_e comb_e * y_e  (2 ACT scale-copies + 2 DVE MACs)
                s = wk.tile([P, DIM], F32, tag="s")
                tmix = wk.tile([P, DIM], F32, tag="tmix")
                nc.scalar.activation(s[:], y_ps[:, 0:256], AF.Copy, scale=comb[:, 0:1])
                nc.scalar.activation(
                    tmix[:], y_ps[:, 256:512], AF.Copy, scale=comb[:, 1:2]
                )
                nc.vector.scalar_tensor_tensor(
                    out=s[:], in0=y_ps[:, 512:768], scalar=comb[:, 2:3], in1=s[:],
                    op0=ALU.mult, op1=ALU.add,
                )
                nc.vector.scalar_tensor_tensor(
                    out=tmix[:], in0=y_ps[:, 768:1024], scalar=comb[:, 3:4], in1=tmix[:],
                    op0=ALU.mult, op1=ALU.add,
                )
                nc.vector.tensor_add(s[:], s[:], tmix[:])

                # scores: 10 unique head-pair dots via slice-muls + grouped reduces
                pbuf = wk.tile([P, 640], F32, tag="pbuf")
                nc.vector.tensor_mul(pbuf[:, 0:192], s[:, 0:192], s[:, 64:256])
                nc.vector.tensor_mul(pbuf[:, 192:320], s[:, 0:128], s[:, 128:256])
                nc.vector.tensor_mul(pbuf[:, 320:384], s[:, 0:64], s[:, 192:256])
                nc.vector.tensor_mul(pbuf[:, 384:640], s[:], s[:])
                sc = wks.tile([P, 16], F32, tag="sc")
                pv = pbuf[:].rearrange("p (n d) -> p n d", d=DH)
                nc.vector.reduce_sum(sc[:, 1:16:5], pv[:, 0:3, :], axis=AX.X)
                nc.vector.reduce_sum(sc[:, 2:12:5], pv[:, 3:5, :], axis=AX.X)
                nc.vector.reduce_sum(sc[:, 3:4], pv[:, 5:6, :], axis=AX.X)
                nc.vector.reduce_sum(sc[:, 0:16:5], pv[:, 6:10, :], axis=AX.X)
                nc.vector.tensor_copy(sc[:, 4:15:5], sc[:, 1:16:5])
                nc.vector.tensor_copy(sc[:, 8:14:5], sc[:, 2:12:5])
                nc.vector.tensor_copy(sc[:, 12:13], sc[:, 3:4])

                # softmax over j (groups of 4), normalization folded into A
                scv = sc[:].rearrange("p (i j) -> p i j", j=H)
                m4 = wks.tile([P, H], F32, tag="m4")
                nc.vector.reduce_max(m4[:], scv, axis=AX.X)
                sub16 = wks.tile([P, 16], F32, tag="sub16")
                nc.vector.tensor_tensor(
                    out=sub16[:].rearrange("p (i j) -> p i j", j=H), in0=scv,
                    in1=m4[:].unsqueeze(2).broadcast_to([P, H, H]), op=ALU.subtract,
                )
                ez16 = wks.tile([P, 16], F32, tag="ez16")
                nc.scalar.activation(ez16[:], sub16[:], AF.Exp, scale=SCALE)
                z4 = wks.tile([P, H], F32, tag="z4")
                nc.vector.reduce_sum(
                    z4[:], ez16[:].rearrange("p (i j) -> p i j", j=H), axis=AX.X
                )
                rz4 = wks.tile([P, H], F32, tag="rz4")
                nc.vector.reciprocal(rz4[:], z4[:])
                a16 = wks.tile([P, 16], F32, tag="a16")
                nc.vector.tensor_tensor(
                    out=a16[:].rearrange("p (i j) -> p i j", j=H),
                    in0=ez16[:].rearrange("p (i j) -> p i j", j=H),
                    in1=rz4[:].unsqueeze(2).broadcast_to([P, H, H]), op=ALU.mult,
                )

                # out_nat[:, i*64:(i+1)*64] = sum_j A[:, 4i+j] * s_j
                acc = wk.tile([P, DIM], F32, tag="acc")
                for i in range(H):
                    nc.vector.tensor_scalar_mul(
                        out=acc[:, ts(i, DH)], in0=s[:, 0:DH],
                        scalar1=a16[:, 4 * i : 4 * i + 1],
                    )
                for j in range(1, H):
                    for i in range(H):
                        nc.vector.scalar_tensor_tensor(
                            out=acc[:, ts(i, DH)], in0=s[:, ts(j, DH)],
                            scalar=a16[:, 4 * i + j : 4 * i + j + 1],
                            in1=acc[:, ts(i, DH)], op0=ALU.mult, op1=ALU.add,
                        )

                # proj: transpose out_nat, matmul with permuted proj_w
                oT_ps = ps_tr.tile([P, 2, P], F32, tag="tr")
                nc.tensor.transpose(oT_ps[:, 0, :], acc[:, 0:P], ident[:])
                nc.tensor.transpose(oT_ps[:, 1, :], acc[:, P:DIM], ident[:])
                oT = wk.tile([P, 2, P], MM_DT, tag="oT")
                nc.scalar.copy(oT[:], oT_ps[:])
                at_ps = ps_at.tile([P, DIM], F32, tag="at")
                for c in range(2):
                    _mm(nc, at_ps[:], oT[:, c, :], projp[:, c, :],
                        start=(c == 0), stop=(c == 1))

                # x2 = x + attn (+proj_b) ; in-place into x tile
                nc.vector.tensor_add(x_t, x_t, at_ps[:])
                if pb_t is not None:
                    nc.vector.tensor_add(x_t, x_t, pb_t[:])

                # LN2 -> h2 -> transpose into resident h2T group tile
                st6b = wks.tile([P, 6], F32, tag="st6")
                nc.vector.bn_stats(st6b[:], x_t)
                mv2 = wks.tile([P, 2], F32, tag="mv")
                nc.vector.bn_aggr(mv2[:], st6b[:])
                rstd2 = wks.tile([P, 1], F32, tag="rstd")
                nc.scalar.activation(rstd2[:], mv2[:, 1:2], AF.Ln, bias=eps_t[:])
                nc.scalar.activation(rstd2[:], rstd2[:], AF.Exp, scale=-0.5)
                h2 = wk.tile([P, DIM], F32, tag="h1")
                nc.vector.tensor_scalar(
                    out=h2[:], in0=x_t, scalar1=mv2[:, 0:1], scalar2=rstd2[:],
                    op0=ALU.subtract, op1=ALU.mult,
                )
                if b2_t is not None:
                    nc.vector.tensor_add(h2[:], h2[:], b2_t[:])
                h2T_ps = ps_tr.tile([P, 2, P], F32, tag="tr")
                nc.tensor.transpose(h2T_ps[:, 0, :], h2[:, 0:P], ident[:])
                nc.tensor.transpose(h2T_ps[:, 1, :], h2[:, P:DIM], ident[:])
                nc.scalar.copy(h2g[t // 2][:, :, ts(t % 2, P)], h2T_ps[:])

        # keep pass B's gelu ACT ops after all pass A ACT ops (one table
        # set switch total instead of per-block thrash)
        tc.strict_bb_all_engine_barrier()

        # ---------------- pass B (MLP) ----------------
        with ExitStack() as bctx:
            bwk = bctx.enter_context(tc.tile_pool(name="bwk", bufs=2))
            owk = bctx.enter_context(tc.tile_pool(name="owk", bufs=3))
            ps_u = bctx.enter_context(tc.tile_pool(name="ps_u", bufs=2, space="PSUM"))
            ps_m = bctx.enter_context(tc.tile_pool(name="ps_m", bufs=2, space="PSUM"))

            for g in range((nt + 1) // 2):
                ntile = min(2, nt - 2 * g)
                ncol = ntile * P
                h2Tg = h2g[g]
                u_sb = bwk.tile([P, 8, 2 * P], MM_DT, tag="u")
                for half in range(2):
                    u_ps = ps_u.tile([P, 4, 2 * P], F32, tag="ups")
                    for mc in range(4):
                        mcc = 4 * half + mc
                        for kc in range(2):
                            _mm(nc, u_ps[:, mc, 0:ncol], fc1w[:, kc, mcc, :],
                                h2Tg[:, kc, 0:ncol], start=(kc == 0), stop=(kc == 1))
                        if f1b_t is not None:
                            nc.scalar.activation(
                                u_sb[:, mcc, 0:ncol], u_ps[:, mc, 0:ncol], AF.Gelu,
                                bias=f1b_t[:, mcc, :],
                            )
                        else:
                            nc.scalar.activation(
                                u_sb[:, mcc, 0:ncol], u_ps[:, mc, 0:ncol], AF.Gelu
                            )
                for tt in range(ntile):
                    t = 2 * g + tt
                    mlp_ps = ps_m.tile([P, DIM], F32, tag="mlp")
                    for kc in range(8):
                        _mm(nc, mlp_ps[:], u_sb[:, kc, ts(tt, P)], fc2w[:, kc, :],
                            start=(kc == 0), stop=(kc == 7))
                    o_sb = owk.tile([P, DIM], F32, tag="o")
                    nc.vector.tensor_add(o_sb[:], xt[t][:], mlp_ps[:])
                    if f2b_t is not None:
                        nc.vector.tensor_add(o_sb[:], o_sb[:], f2b_t[:])
                    nc.sync.dma_start(out=ov[t], in_=o_sb[:])

    nc.compile()
    return nc


def preprocess(inputs):
    """Host-side tiny-weight preprocessing (all O(DIM^2))."""
    g1 = inputs["norm1_g"].astype(np.float32)
    b1 = inputs["norm1_b"].astype(np.float32)
    g2 = inputs["norm2_g"].astype(np.float32)
    b2 = inputs["norm2_b"].astype(np.float32)
    qkv_w = np.asarray(inputs["qkv_w"], np.float32)
    gate_w = np.asarray(inputs["gate_w"], np.float32)
    gate_b = np.asarray(inputs["gate_b"], np.float32)
    proj_w = np.asarray(inputs["proj_w"], np.float32)
    proj_b = np.asarray(inputs["proj_b"], np.float32)
    fc1_w = np.asarray(inputs["fc1_w"], np.float32)
    fc1_b = np.asarray(inputs["fc1_b"], np.float32)
    fc2_w = np.asarray(inputs["fc2_w"], np.float32)
    fc2_b = np.asarray(inputs["fc2_b"], np.float32)

    # fold q+k+v (they alias one buffer in the reference) and the LN1 gain
    wfold = qkv_w[:, :, 0:256] + qkv_w[:, :, 256:512] + qkv_w[:, :, 512:768]
    wq = g1[None, :, None] * wfold                       # [E, 256, 256]
    gw = g1[:, None] * gate_w                            # [256, E]
    catw = np.concatenate(
        [wq.transpose(1, 0, 2).reshape(DIM, E * DIM), gw], axis=1
    )                                                    # [256, 1028]
    catw = np.ascontiguousarray(catw.reshape(2, P, 1028))

    # proj permutation: out_flat[dh*4+h] = out[h, dh] -> rows reordered
    projp = proj_w.reshape(DH, H, DIM).transpose(1, 0, 2).reshape(DIM, DIM)
    projp = np.ascontiguousarray(projp.reshape(2, P, DIM))

    fc1p = g2[:, None] * fc1_w                           # [256, 1024]
    fc1w = np.ascontiguousarray(
        fc1p.reshape(2, P, 8, P).transpose(0, 2, 1, 3)
    )                                                    # [2, 8, 128, 128]
    fc2w = np.ascontiguousarray(fc2_w.reshape(8, P, DIM))

    flags = {
        "b1": bool(np.any(b1)),
        "b2": bool(np.any(b2)),
        "pb": bool(np.any(proj_b)),
        "f2b": bool(np.any(fc2_b)),
        "gb": bool(np.any(gate_b)),
        "f1b": bool(np.any(fc1_b)),
    }
    wmap = {"catw": catw, "projp": projp, "fc1w": fc1w, "fc2w": fc2w}
    if flags["b1"]:
        wmap["b1row"] = b1.reshape(1, DIM)
    if flags["b2"]:
        wmap["b2row"] = b2.reshape(1, DIM)
    if flags["pb"]:
        wmap["pbrow"] = proj_b.reshape(1, DIM)
    if flags["f2b"]:
        wmap["f2brow"] = fc2_b.reshape(1, DIM)
    if flags["gb"]:
        wmap["gbrow"] = gate_b.reshape(1, E)
    if flags["f1b"]:
        wmap["f1bcol"] = fc1_b.reshape(HID, 1)
    return wmap, flags


_cache = {}


def _get_program(nt, flags):
    key = (nt, tuple(sorted(flags.items())), str(MM_DT))
    if key not in _cache:
        _cache[key] = build_program(nt, flags)
    return _cache[key]


def kernel(**inputs):
    x = np.asarray(inputs["x"], np.float32)
    n = x.shape[0]
    assert n % NCORES == 0
    ntok = n // NCORES
    assert ntok % P == 0
    nt = ntok // P

    wmap, flags = preprocess(inputs)
    nc = _get_program(nt, flags)

    in_maps = []
    for c in range(NCORES):
        m = dict(wmap)
        m["x"] = np.ascontiguousarray(x[c * ntok : (c + 1) * ntok])
        in_maps.append(m)

    trace = bool(int(os.environ.get("KERNEL_TRACE", "0")))
    res = run_bass_kernel_spmd(nc, in_maps, core_ids=list(range(NCORES)), trace=trace)
    out = np.concatenate([r["out"] for r in res.results], axis=0)
    if trace and res.exec_time_ns is not None:
        print(f"HW exec time: {res.exec_time_ns} ns")
        kernel.last_exec_time_ns = res.exec_time_ns
        kernel.last_trace = res.instructions_and_trace
    return out


# revision 13
# speedup vs baseline: 2.0795x; 2.0795x over previous
"""Trainium2 Bass kernel for the MEMOL MoE-attention block (nn_MEMOL_48052094107931).

Computation (per token row x of length 256):
  h1 = LN(x, g1, b1)
  gate = softmax(h1 @ gate_w + gate_b); comb = top2-masked gate probs (not renorm.)
  s  = sum_e comb_e * (h1 @ Wfold_e)           # qkv folded: q=k=v alias => one 256x256/expert
  A  = softmax(scale * s s^T) per token (4x4 over heads)
  out_nat[h*64+d] = (A s)[h, d]                # torch transpose(1,2).reshape folded into proj
  x2 = x + out_nat @ proj_w_perm + proj_b
  y  = x2 + gelu(LN(x2,g2,b2) @ fc1_w + fc1_b) @ fc2_w + fc2_b

Strategy: pure data-parallel over the batch across 8 NeuronCores (no collectives).
One NEFF, SPMD via run_bass_kernel_spmd. Token-major layout on chip; activations
transposed on the PE (with identity) where a matmul needs them as stationary lhsT.
Matmuls run as float32r (PE fast-fp32 mode, ~1.5e-4 rel) EXCEPT the gate logits,
which stay full fp32: top-2 expert selection is discontinuous, so logit noise
flips expert assignments and produces O(1) output errors.
LN rstd uses a bit-trick + Newton rsqrt on the Vector engine so the Scalar engine
only ever needs the exp table set in pass A (gelu set loads once in pass B).
"""

import os
from contextlib import ExitStack

import numpy as np

import concourse.bass as bass
import concourse.mybir as mybir
import concourse.tile as tile
from concourse import bacc
from concourse.bass import ts
from concourse.bass_utils import run_bass_kernel_spmd
from concourse.masks import make_identity

F32 = mybir.dt.float32
F32R = mybir.dt.float32r
I32 = mybir.dt.int32
AF = mybir.ActivationFunctionType
ALU = mybir.AluOpType
AX = mybir.AxisListType

DIM = 256
E = 4
H = 4
DH = 64
HID = 1024
SCALE = DH ** -0.5
NCORES = 8
P = 128
EPS = 1e-5
G = 4  # tiles per group

MM_DT = F32 if os.environ.get("KERNEL_MM_F32", "0") == "1" else F32R
RSQRT_MAGIC = 0x5F3759DF


def _mm(nc, out, lhsT, rhs, **kw):
    nc.tensor.matmul(out, lhsT.bitcast(MM_DT), rhs.bitcast(MM_DT), **kw)


def _newton_rsqrt(nc, pool, var_ap, eps_imm, n, tag):
    """rstd[128, n] = 1/sqrt(var + eps) via fast-inverse-sqrt seed + 3 Newton
    steps on the Vector engine (avoids the Ln/Sqrt ACT table sets entirely).
    var_ap: [128, n] fp32 AP. Returns the rstd tile."""
    va = pool.tile([P, n], F32, tag=f"{tag}va", name=f"{tag}va")
    nc.vector.tensor_scalar(out=va[:], in0=var_ap, scalar1=eps_imm, scalar2=None,
                            op0=ALU.add)
    y = pool.tile([P, n], F32, tag=f"{tag}y", name=f"{tag}y")
    # y0 bits: magic - (va_bits >> 1); (magic - x) = bitwise_not(x - magic) + 1
    # avoided by using scalar_tensor_tensor: (va>>1)*(-1) + magic
    yi = y[:].bitcast(I32)
    nc.vector.tensor_scalar(out=yi, in0=va[:].bitcast(I32), scalar1=1, scalar2=None,
                            op0=ALU.arith_shift_right)
    # yi = magic - yi  ==  (yi * -1) + magic, fused int tensor_scalar
    nc.vector.tensor_scalar(out=yi, in0=yi, scalar1=-1, scalar2=RSQRT_MAGIC,
                            op0=ALU.mult, op1=ALU.add)
    tmp = pool.tile([P, n], F32, tag=f"{tag}t", name=f"{tag}t")
    for _ in range(3):
        nc.vector.tensor_mul(tmp[:], y[:], y[:])
        nc.vector.scalar_tensor_tensor(out=tmp[:], in0=tmp[:], scalar=-0.5,
                                       in1=va[:], op0=ALU.mult, op1=ALU.mult)
        nc.vector.tensor_scalar(out=tmp[:], in0=tmp[:], scalar1=1.5, scalar2=None,
                                op0=ALU.add)
        nc.vector.tensor_mul(y[:], y[:], tmp[:])
    return y


def build_program(nt, flags):
    """Per-core program: nt tiles of 128 tokens."""
    assert nt % G == 0
    ng = nt // G
    ntok = nt * P
    nc = bacc.Bacc(
        "TRN2", target_bir_lowering=False, debug=False, enable_asserts=False
    )

    x_d = nc.dram_tensor("x", [ntok, DIM], F32, kind="ExternalInput").ap()
    wq_d = nc.dram_tensor("wq", [2, P, HID], F32, kind="ExternalInput").ap()
    gw_d = nc.dram_tensor("gw", [2, P, E], F32, kind="ExternalInput").ap()
    projp_d = nc.dram_tensor("projp", [2, P, DIM], F32, kind="ExternalInput").ap()
    fc1w_d = nc.dram_tensor("fc1w", [2, 8, P, P], F32, kind="ExternalInput").ap()
    fc2w_d = nc.dram_tensor("fc2w", [8, P, DIM], F32, kind="ExternalInput").ap()
    out_d = nc.dram_tensor("out", [ntok, DIM], F32, kind="ExternalOutput").ap()
    b1_d = b2_d = pb_d = f2b_d = gb_d = f1b_d = None
    if flags["b1"]:
        b1_d = nc.dram_tensor("b1row", [1, DIM], F32, kind="ExternalInput").ap()
    if flags["b2"]:
        b2_d = nc.dram_tensor("b2row", [1, DIM], F32, kind="ExternalInput").ap()
    if flags["pb"]:
        pb_d = nc.dram_tensor("pbrow", [1, DIM], F32, kind="ExternalInput").ap()
    if flags["f2b"]:
        f2b_d = nc.dram_tensor("f2brow", [1, DIM], F32, kind="ExternalInput").ap()
    if flags["gb"]:
        gb_d = nc.dram_tensor("gbrow", [1, E], F32, kind="ExternalInput").ap()
    if flags["f1b"]:
        f1b_d = nc.dram_tensor("f1bcol", [HID, 1], F32, kind="ExternalInput").ap()

    xv = x_d.rearrange("(n p) d -> n p d", p=P)
    ov = out_d.rearrange("(n p) d -> n p d", p=P)

    with tile.TileContext(nc) as tc, ExitStack() as ctx:
        singles = ctx.enter_context(tc.tile_pool(name="singles", bufs=1))

        ident = singles.tile([P, P], F32)
        make_identity(nc, ident[:])

        gwt = singles.tile([P, 2, E], F32)  # gate weights stay full fp32
        nc.sync.dma_start(out=gwt[:], in_=gw_d.transpose([1, 0, 2]))
        wq = singles.tile([P, 2, HID], MM_DT)
        projp = singles.tile([P, 2, DIM], MM_DT)
        fc1w = singles.tile([P, 2, 8, P], MM_DT)
        fc2w = singles.tile([P, 8, DIM], MM_DT)
        with tc.tile_pool(name="wstage", bufs=1) as wstage:
            for (dst, src) in [
                (wq, wq_d.transpose([1, 0, 2])),
                (projp, projp_d.transpose([1, 0, 2])),
                (fc1w, fc1w_d.transpose([2, 0, 1, 3])),
                (fc2w, fc2w_d.transpose([1, 0, 2])),
            ]:
                stg = wstage.tile(list(dst.shape), F32, tag="stg", name="stg")
                nc.sync.dma_start(out=stg[:], in_=src)
                nc.vector.tensor_copy(dst[:], stg[:])

        def brow(name, d_ap, width):
            t = singles.tile([P, width], F32, name=name)
            nc.sync.dma_start(out=t[:], in_=d_ap.partition_broadcast(P).squeeze(1))
            return t

        b1_t = brow("b1", b1_d, DIM) if flags["b1"] else None
        b2_t = brow("b2", b2_d, DIM) if flags["b2"] else None
        pb_t = brow("pb", pb_d, DIM) if flags["pb"] else None
        f2b_t = brow("f2b", f2b_d, DIM) if flags["f2b"] else None
        gb_t = brow("gb", gb_d, E) if flags["gb"] else None
        f1b_t = None
        if flags["f1b"]:
            f1b_t = singles.tile([P, 8, 1], F32)
            nc.sync.dma_start(
                out=f1b_t[:], in_=f1b_d.rearrange("(c p) o -> p c o", p=P)
            )

        xpool = ctx.enter_context(tc.tile_pool(name="xtiles", bufs=nt))
        h2pool = ctx.enter_context(tc.tile_pool(name="h2tiles", bufs=ng))
        xt = []
        for t in range(nt):
            xt.append(xpool.tile([P, DIM], F32, tag="x", name=f"x{t}"))
            nc.sync.dma_start(out=xt[t][:], in_=xv[t])
        h2g = [
            h2pool.tile([P, 2, G * P], MM_DT, tag="h2", name=f"h2g{g}")
            for g in range(ng)
        ]

        # ---------------- pass A ----------------
        with ExitStack() as actx:
            wk = actx.enter_context(tc.tile_pool(name="wk", bufs=3))
            wkt = actx.enter_context(tc.tile_pool(name="wkt", bufs=G + 1))
            wkg = actx.enter_context(tc.tile_pool(name="wkg", bufs=2))
            wkb = actx.enter_context(tc.tile_pool(name="wkb", bufs=1))
            wks = actx.enter_context(tc.tile_pool(name="wks", bufs=4))
            ps_tr = actx.enter_context(tc.tile_pool(name="ps_tr", bufs=2, space="PSUM"))
            ps_y = actx.enter_context(tc.tile_pool(name="ps_y", bufs=2, space="PSUM"))
            ps_z = actx.enter_context(tc.tile_pool(name="ps_z", bufs=1, space="PSUM"))
            ps_at = actx.enter_context(tc.tile_pool(name="ps_at", bufs=1, space="PSUM"))

            for g in range(ng):
                tls = [g * G + tt for tt in range(G)]
                # --- LN1 stats (per tile) + batched Newton rstd ---
                mv4 = wkg.tile([P, G, 2], F32, tag="mv4")
                for tt, t in enumerate(tls):
                    st6 = wks.tile([P, 6], F32, tag="st6", name="st6")
                    nc.vector.bn_stats(st6[:], xt[t][:])
                    nc.vector.bn_aggr(mv4[:, tt, :], st6[:])
                rstd4 = _newton_rsqrt(nc, wkg, mv4[:, :, 1], EPS, G, "r1")

                h1T_l, h1Tf_l = [], []
                for tt, t in enumerate(tls):
                    h1 = wk.tile([P, DIM], F32, tag="h1", name="h1")
                    nc.vector.tensor_scalar(
                        out=h1[:], in0=xt[t][:], scalar1=mv4[:, tt, 0:1],
                        scalar2=rstd4[:, tt : tt + 1],
                        op0=ALU.subtract, op1=ALU.mult,
                    )
                    if b1_t is not None:
                        nc.vector.tensor_add(h1[:], h1[:], b1_t[:])
                    h1T_ps = ps_tr.tile([P, 2, P], F32, tag="tr", name="h1Tps")
                    nc.tensor.transpose(h1T_ps[:, 0, :], h1[:, 0:P], ident[:])
                    nc.tensor.transpose(h1T_ps[:, 1, :], h1[:, P:DIM], ident[:])
                    h1T = wkt.tile([P, 2, P], MM_DT, tag="h1T", name="h1T")
                    nc.scalar.copy(h1T[:], h1T_ps[:])
                    h1Tf = wkt.tile([P, 2, P], F32, tag="h1Tf", name="h1Tf")
                    nc.vector.tensor_copy(h1Tf[:], h1T_ps[:])
                    h1T_l.append(h1T)
                    h1Tf_l.append(h1Tf)

                # --- gate logit matmuls (full fp32) ---
                z4_ps = ps_z.tile([P, G, E], F32, tag="z4", name="z4ps")
                for tt in range(G):
                    for c in range(2):
                        nc.tensor.matmul(
                            z4_ps[:, tt, :], h1Tf_l[tt][:, c, :], gwt[:, c, :],
                            start=(c == 0), stop=(c == 1),
                        )

                # --- gate: batched softmax + top-2 mask over the group ---
                if gb_t is not None:
                    nc.vector.tensor_tensor(
                        out=z4_ps[:], in0=z4_ps[:],
                        in1=gb_t[:].unsqueeze(1).broadcast_to([P, G, E]), op=ALU.add,
                    )
                z4 = wkg.tile([P, G, E], F32, tag="z4s", name="z4s")
                nc.vector.tensor_copy(z4[:], z4_ps[:])
                m1 = wkg.tile([P, G], F32, tag="m1", name="m1")
                nc.vector.reduce_max(m1[:], z4[:], axis=AX.X)
                m1b = m1[:].unsqueeze(2).broadcast_to([P, G, E])
                keq = wkg.tile([P, G, E], F32, tag="keq", name="keq")
                nc.vector.tensor_tensor(out=keq[:], in0=z4[:], in1=m1b, op=ALU.is_equal)
                zmsk = wkg.tile([P, G, E], F32, tag="zmsk", name="zmsk")
                nc.vector.scalar_tensor_tensor(
                    out=zmsk[:], in0=keq[:], scalar=-1e30, in1=z4[:],
                    op0=ALU.mult, op1=ALU.add,
                )
                m2 = wkg.tile([P, G], F32, tag="m2", name="m2")
                nc.vector.reduce_max(m2[:], zmsk[:], axis=AX.X)
                zs = wkg.tile([P, G, E], F32, tag="zs", name="zs")
                nc.vector.tensor_tensor(out=zs[:], in0=z4[:], in1=m1b, op=ALU.subtract)
                ez = wkg.tile([P, G, E], F32, tag="ez", name="ez")
                nc.scalar.activation(ez[:], zs[:], AF.Exp)
                zsum = wkg.tile([P, G], F32, tag="zsum", name="zsum")
                nc.vector.reduce_sum(zsum[:], ez[:], axis=AX.X)
                rzg = wkg.tile([P, G], F32, tag="rzg", name="rzg")
                nc.vector.reciprocal(rzg[:], zsum[:])
                keep = wkg.tile([P, G, E], F32, tag="keep", name="keep")
                nc.vector.tensor_tensor(
                    out=keep[:], in0=z4[:],
                    in1=m2[:].unsqueeze(2).broadcast_to([P, G, E]), op=ALU.is_ge,
                )
                comb = wkg.tile([P, G, E], F32, tag="comb", name="comb")
                nc.vector.tensor_tensor(
                    out=comb[:], in0=ez[:],
                    in1=rzg[:].unsqueeze(2).broadcast_to([P, G, E]), op=ALU.mult,
                )
                nc.vector.tensor_tensor(out=comb[:], in0=comb[:], in1=keep[:], op=ALU.mult)

                # --- qkv matmuls + expert mix: s4[:, tt, :] = sum_e comb_e*y_e ---
                s4 = wkg.tile([P, G, DIM], F32, tag="s4", name="s4")
                for tt in range(G):
                    y_ps = ps_y.tile([P, HID], F32, tag="y", name="yps")
                    for c in range(2):
                        kw = dict(start=(c == 0), stop=(c == 1))
                        _mm(nc, y_ps[:, 0:512], h1T_l[tt][:, c, :], wq[:, c, 0:512], **kw)
                        _mm(nc, y_ps[:, 512:1024], h1T_l[tt][:, c, :],
                            wq[:, c, 512:1024], **kw)
                    tm = wks.tile([P, DIM], F32, tag="tm", name="tm")
                    nc.scalar.activation(
                        s4[:, tt, :], y_ps[:, 0:256], AF.Copy,
                        scale=comb[:, tt, 0:1],
                    )
                    nc.scalar.activation(
                        tm[:], y_ps[:, 256:512], AF.Copy, scale=comb[:, tt, 1:2]
                    )
                    nc.vector.scalar_tensor_tensor(
                        out=s4[:, tt, :], in0=y_ps[:, 512:768],
                        scalar=comb[:, tt, 2:3], in1=s4[:, tt, :],
                        op0=ALU.mult, op1=ALU.add,
                    )
                    nc.vector.scalar_tensor_tensor(
                        out=tm[:], in0=y_ps[:, 768:1024], scalar=comb[:, tt, 3:4],
                        in1=tm[:], op0=ALU.mult, op1=ALU.add,
                    )
                    nc.vector.tensor_add(s4[:, tt, :], s4[:, tt, :], tm[:])

                # --- attention scores (10 unique pairs), batched over group ---
                pb4 = wkb.tile([P, G, 640], F32, tag="pb4", name="pb4")
                nc.vector.tensor_tensor(
                    out=pb4[:, :, 0:192], in0=s4[:, :, 0:192], in1=s4[:, :, 64:256],
                    op=ALU.mult)
                nc.vector.tensor_tensor(
                    out=pb4[:, :, 192:320], in0=s4[:, :, 0:128], in1=s4[:, :, 128:256],
                    op=ALU.mult)
                nc.vector.tensor_tensor(
                    out=pb4[:, :, 320:384], in0=s4[:, :, 0:64], in1=s4[:, :, 192:256],
                    op=ALU.mult)
                nc.vector.tensor_tensor(
                    out=pb4[:, :, 384:640], in0=s4[:], in1=s4[:], op=ALU.mult)
                sc4 = wkg.tile([P, G, 16], F32, tag="sc4", name="sc4")
                pv = pb4[:].rearrange("p g (n d) -> p g n d", d=DH)
                nc.vector.reduce_sum(sc4[:, :, 1:16:5], pv[:, :, 0:3, :], axis=AX.X)
                nc.vector.reduce_sum(sc4[:, :, 2:12:5], pv[:, :, 3:5, :], axis=AX.X)
                nc.vector.reduce_sum(sc4[:, :, 3:4], pv[:, :, 5:6, :], axis=AX.X)
                nc.vector.reduce_sum(sc4[:, :, 0:16:5], pv[:, :, 6:10, :], axis=AX.X)
                nc.vector.tensor_copy(sc4[:, :, 4:15:5], sc4[:, :, 1:16:5])
                nc.vector.tensor_copy(sc4[:, :, 8:14:5], sc4[:, :, 2:12:5])
                nc.vector.tensor_copy(sc4[:, :, 12:13], sc4[:, :, 3:4])

                # --- softmax over j; normalization folded into A ---
                scv = sc4[:].rearrange("p g (i j) -> p g i j", j=H)
                m4 = wkg.tile([P, G, H], F32, tag="m4", name="m4")
                nc.vector.reduce_max(m4[:], scv, axis=AX.X)
                sub4 = wkg.tile([P, G, 16], F32, tag="sub4", name="sub4")
                nc.vector.tensor_tensor(
                    out=sub4[:].rearrange("p g (i j) -> p g i j", j=H), in0=scv,
                    in1=m4[:].unsqueeze(3).broadcast_to([P, G, H, H]), op=ALU.subtract,
                )
                ez4 = wkg.tile([P, G, 16], F32, tag="ez4", name="ez4")
                nc.scalar.activation(ez4[:], sub4[:], AF.Exp, scale=SCALE)
                zs4 = wkg.tile([P, G, H], F32, tag="zs4", name="zs4")
                nc.vector.reduce_sum(
                    zs4[:], ez4[:].rearrange("p g (i j) -> p g i j", j=H), axis=AX.X
                )
                rz4 = wkg.tile([P, G, H], F32, tag="rz4", name="rz4")
                nc.vector.reciprocal(rz4[:], zs4[:])
                a4 = wkg.tile([P, G, 16], F32, tag="a4", name="a4")
                nc.vector.tensor_tensor(
                    out=a4[:].rearrange("p g (i j) -> p g i j", j=H),
                    in0=ez4[:].rearrange("p g (i j) -> p g i j", j=H),
                    in1=rz4[:].unsqueeze(3).broadcast_to([P, G, H, H]), op=ALU.mult,
                )

                # --- out mix + proj + residual + LN2 (per tile) ---
                mv4b = wkg.tile([P, G, 2], F32, tag="mv4b")
                h2s = []
                for tt, t in enumerate(tls):
                    acc = wk.tile([P, DIM], F32, tag="acc", name="acc")
                    for i in range(H):
                        nc.vector.tensor_scalar_mul(
                            out=acc[:, ts(i, DH)], in0=s4[:, tt, 0:DH],
                            scalar1=a4[:, tt, 4 * i : 4 * i + 1],
                        )
                    for j in range(1, H):
                        for i in range(H):
                            nc.vector.scalar_tensor_tensor(
                                out=acc[:, ts(i, DH)], in0=s4[:, tt, ts(j, DH)],
                                scalar=a4[:, tt, 4 * i + j : 4 * i + j + 1],
                                in1=acc[:, ts(i, DH)], op0=ALU.mult, op1=ALU.add,
                            )
                    oT_ps = ps_tr.tile([P, 2, P], F32, tag="tr", name="oTps")
                    nc.tensor.transpose(oT_ps[:, 0, :], acc[:, 0:P], ident[:])
                    nc.tensor.transpose(oT_ps[:, 1, :], acc[:, P:DIM], ident[:])
                    oT = wk.tile([P, 2, P], MM_DT, tag="oT", name="oT")
                    nc.scalar.copy(oT[:], oT_ps[:])
                    at_ps = ps_at.tile([P, DIM], F32, tag="at", name="atps")
                    for c in range(2):
                        _mm(nc, at_ps[:], oT[:, c, :], projp[:, c, :],
                            start=(c == 0), stop=(c == 1))
                    nc.vector.tensor_add(xt[t][:], xt[t][:], at_ps[:])
                    if pb_t is not None:
                        nc.vector.tensor_add(xt[t][:], xt[t][:], pb_t[:])
                    st6b = wks.tile([P, 6], F32, tag="st6", name="st6b")
                    nc.vector.bn_stats(st6b[:], xt[t][:])
                    nc.vector.bn_aggr(mv4b[:, tt, :], st6b[:])

                rstd4b = _newton_rsqrt(nc, wkg, mv4b[:, :, 1], EPS, G, "r2")
                for tt, t in enumerate(tls):
                    h2 = wk.tile([P, DIM], F32, tag="h1", name="h2")
                    nc.vector.tensor_scalar(
                        out=h2[:], in0=xt[t][:], scalar1=mv4b[:, tt, 0:1],
                        scalar2=rstd4b[:, tt : tt + 1],
                        op0=ALU.subtract, op1=ALU.mult,
                    )
                    if b2_t is not None:
                        nc.vector.tensor_add(h2[:], h2[:], b2_t[:])
                    h2T_ps = ps_tr.tile([P, 2, P], F32, tag="tr", name="h2Tps")
                    nc.tensor.transpose(h2T_ps[:, 0, :], h2[:, 0:P], ident[:])
                    nc.tensor.transpose(h2T_ps[:, 1, :], h2[:, P:DIM], ident[:])
                    nc.scalar.copy(h2g[g][:, :, ts(tt, P)], h2T_ps[:])

        # single ACT table-set switch: all gelu after all exp
        tc.strict_bb_all_engine_barrier()

        # ---------------- pass B (MLP) ----------------
        with ExitStack() as bctx:
            bwk = bctx.enter_context(tc.tile_pool(name="bwk", bufs=2))
            owk = bctx.enter_context(tc.tile_pool(name="owk", bufs=3))
            ps_u = bctx.enter_context(tc.tile_pool(name="ps_u", bufs=1, space="PSUM"))
            ps_m = bctx.enter_context(tc.tile_pool(name="ps_m", bufs=2, space="PSUM"))

            for g in range(ng):
                ncol = G * P
                h2Tg = h2g[g]
                u_sb = bwk.tile([P, 8, ncol], MM_DT, tag="u", name="u_sb")
                for half in range(2):
                    u_ps = ps_u.tile([P, 4, ncol], F32, tag="ups", name="u_ps")
                    for mc in range(4):
                        mcc = 4 * half + mc
                        for kc in range(2):
                            _mm(nc, u_ps[:, mc, :], fc1w[:, kc, mcc, :],
                                h2Tg[:, kc, :], start=(kc == 0), stop=(kc == 1))
                        if f1b_t is not None:
                            nc.scalar.activation(
                                u_sb[:, mcc, :], u_ps[:, mc, :], AF.Gelu,
                                bias=f1b_t[:, mcc, :],
                            )
                        else:
                            nc.scalar.activation(u_sb[:, mcc, :], u_ps[:, mc, :], AF.Gelu)
                for tt in range(G):
                    t = g * G + tt
                    mlp_ps = ps_m.tile([P, DIM], F32, tag="mlp", name="mlp_ps")
                    for kc in range(8):
                        _mm(nc, mlp_ps[:], u_sb[:, kc, ts(tt, P)], fc2w[:, kc, :],
                            start=(kc == 0), stop=(kc == 7))
                    o_sb = owk.tile([P, DIM], F32, tag="o", name="o_sb")
                    nc.vector.tensor_add(o_sb[:], xt[t][:], mlp_ps[:])
                    if f2b_t is not None:
                        nc.vector.tensor_add(o_sb[:], o_sb[:], f2b_t[:])
                    nc.sync.dma_start(out=ov[t], in_=o_sb[:])

    nc.compile()
    return nc


def preprocess(inputs):
    """Host-side tiny-weight preprocessing (all O(DIM^2))."""
    g1 = inputs["norm1_g"].astype(np.float32)
    b1 = inputs["norm1_b"].astype(np.float32)
    g2 = inputs["norm2_g"].astype(np.float32)
    b2 = inputs["norm2_b"].astype(np.float32)
    qkv_w = np.asarray(inputs["qkv_w"], np.float32)
    gate_w = np.asarray(inputs["gate_w"], np.float32)
    gate_b = np.asarray(inputs["gate_b"], np.float32)
    proj_w = np.asarray(inputs["proj_w"], np.float32)
    proj_b = np.asarray(inputs["proj_b"], np.float32)
    fc1_w = np.asarray(inputs["fc1_w"], np.float32)
    fc1_b = np.asarray(inputs["fc1_b"], np.float32)
    fc2_w = np.asarray(inputs["fc2_w"], np.float32)
    fc2_b = np.asarray(inputs["fc2_b"], np.float32)

    # fold q+k+v (they alias one buffer in the reference) and the LN1 gain
    wfold = qkv_w[:, :, 0:256] + qkv_w[:, :, 256:512] + qkv_w[:, :, 512:768]
    wqm = g1[None, :, None] * wfold                      # [E, 256, 256]
    wq = np.ascontiguousarray(
        wqm.transpose(1, 0, 2).reshape(DIM, E * DIM).reshape(2, P, E * DIM)
    )
    gw = np.ascontiguousarray((g1[:, None] * gate_w).reshape(2, P, E))

    projp = proj_w.reshape(DH, H, DIM).transpose(1, 0, 2).reshape(DIM, DIM)
    projp = np.ascontiguousarray(projp.reshape(2, P, DIM))

    fc1p = g2[:, None] * fc1_w
    fc1w = np.ascontiguousarray(fc1p.reshape(2, P, 8, P).transpose(0, 2, 1, 3))
    fc2w = np.ascontiguousarray(fc2_w.reshape(8, P, DIM))

    flags = {
        "b1": bool(np.any(b1)),
        "b2": bool(np.any(b2)),
        "pb": bool(np.any(proj_b)),
        "f2b": bool(np.any(fc2_b)),
        "gb": bool(np.any(gate_b)),
        "f1b": bool(np.any(fc1_b)),
    }
    wmap = {"wq": wq, "gw": gw, "projp": projp, "fc1w": fc1w, "fc2w": fc2w}
    if flags["b1"]:
        wmap["b1row"] = b1.reshape(1, DIM)
    if flags["b2"]:
        wmap["b2row"] = b2.reshape(1, DIM)
    if flags["pb"]:
        wmap["pbrow"] = proj_b.reshape(1, DIM)
    if flags["f2b"]:
        wmap["f2brow"] = fc2_b.reshape(1, DIM)
    if flags["gb"]:
        wmap["gbrow"] = gate_b.reshape(1, E)
    if flags["f1b"]:
        wmap["f1bcol"] = fc1_b.reshape(HID, 1)
    return wmap, flags


_cache = {}


def _get_program(nt, flags):
    key = (nt, tuple(sorted(flags.items())), str(MM_DT))
    if key not in _cache:
        _cache[key] = build_program(nt, flags)
    return _cache[key]


def kernel(**inputs):
    x = np.asarray(inputs["x"], np.float32)
    n = x.shape[0]
    assert n % NCORES == 0
    ntok = n // NCORES
    assert ntok % P == 0
    nt = ntok // P

    wmap, flags = preprocess(inputs)
    nc = _get_program(nt, flags)

    in_maps = []
    for c in range(NCORES):
        m = dict(wmap)
        m["x"] = np.ascontiguousarray(x[c * ntok : (c + 1) * ntok])
        in_maps.append(m)

    trace = bool(int(os.environ.get("KERNEL_TRACE", "0")))
    res = run_bass_kernel_spmd(nc, in_maps, core_ids=list(range(NCORES)), trace=trace)
    out = np.concatenate([r["out"] for r in res.results], axis=0)
    if trace and res.exec_time_ns is not None:
        print(f"HW exec time: {res.exec_time_ns} ns")
        kernel.last_exec_time_ns = res.exec_time_ns
        kernel.last_trace = res.instructions_and_trace
    return out
